# revision 4
# baseline (speedup 1.0000x reference)
"""CommutatorConv2d kernel for Trainium2 (Bass/Tile), 8-core data-parallel.

Math: the reference's commutator/anticommutator conv reduces exactly to a
single-channel 3x3 conv on the channel-summed input:

    out[b] = T @ xs[b] @ A + Bm @ xs[b] @ T + bias,   xs = x.sum(axis=1)

where T is the 128x128 tridiagonal-ones matrix and A, Bm are tridiagonal
matrices built from K's column/row sums scaled by (lambda_c +/- lambda_a):
sum_{i,m} XK[...,i,m] = sum_{i,j} patch[i,j]*colsum(K)[j] and
sum_{j,i} KX[...,j,i] = sum_{m,i} patch[m,i]*rowsum(K)[m], so the effective
3x3 kernel is W[i,j] = a[j] + b[i], separable into a row-conv on the vertical
boxsum plus a col-conv on the horizontal boxsum = the two matrix sandwiches.

Layout: each core's batch shard is handed to the device as [H, B_loc, C, W]
(h-major) so every SBUF partition receives one long contiguous DRAM run per
DMA — 8KB descriptors instead of 512B ones, which is the difference between
~170 GB/s and ~358 GB/s on the HBM path. The device still streams the full
shard HBM->SBUF.

Per core (2 batches x 2 channel-halves): load half -> DVE strided reduce_sum
over c -> accumulate xs_half.T @ [T | Bm.T] into PSUM (the half-combine rides
the matmul accumulation) -> out = uv0.T @ A + uv1.T @ T in PSUM -> bias-add
fused into the PSUM->SBUF copy on DVE -> store. Loads ride the sync HWDGE
ring, stores the scalar ring, tiny constants the gpsimd SWDGE queue.
"""

import numpy as np

B, C, H, W = 16, 32, 128, 128
N_CORES = 8
B_LOC = B // N_CORES
HALF = C // 2

_PROGRAM = None
LAST_RESULTS = None


def _build_program():
    import concourse.mybir as mybir
    from concourse import bacc
    from concourse.bass import MemorySpace
    from concourse.tile import TileContext

    f32 = mybir.dt.float32
    nc = bacc.Bacc(
        "TRN2", target_bir_lowering=False, debug=False, num_devices=N_CORES
    )

    x_dram = nc.dram_tensor("x", (H, B_LOC, C, W), f32, kind="ExternalInput")
    a_dram = nc.dram_tensor("amat", (H, W), f32, kind="ExternalInput")
    t_dram = nc.dram_tensor("tmat", (H, W), f32, kind="ExternalInput")
    tbm_dram = nc.dram_tensor("tbm", (H, 2 * W), f32, kind="ExternalInput")
    bias_dram = nc.dram_tensor("biascol", (H, 1), f32, kind="ExternalInput")
    out_dram = nc.dram_tensor("out", (B_LOC, H, W), f32, kind="ExternalOutput")

    x_ap = x_dram.ap()
    out_ap = out_dram.ap()

    with TileContext(nc) as tc:
        with (
            tc.tile_pool(name="consts", bufs=1) as cpool,
            tc.tile_pool(name="xpool", bufs=2) as xpool,
            tc.tile_pool(name="xspool", bufs=2) as xspool,
            tc.tile_pool(name="uvpool", bufs=2) as uvpool,
            tc.tile_pool(name="opool", bufs=2) as opool,
            tc.tile_pool(name="psum", bufs=2, space=MemorySpace.PSUM) as ppool,
        ):
            # Bulk x loads first (sync ring) so data streams immediately.
            halves = []  # [b][s] -> [H, HALF*W] tile
            for b in range(B_LOC):
                pair = []
                for s in range(2):
                    xh = xpool.tile([H, HALF * W], f32, tag=f"xh{s}")
                    nc.sync.dma_start(
                        out=xh.rearrange("h (c w) -> h c w", w=W),
                        in_=x_ap[:, b, s * HALF : (s + 1) * HALF, :],
                    )
                    pair.append(xh)
                halves.append(pair)

            # Tiny constants via the gpsimd SWDGE queue (off the HWDGE rings).
            a_sb = cpool.tile([H, W], f32)
            nc.gpsimd.dma_start(out=a_sb, in_=a_dram.ap())
            t_sb = cpool.tile([H, W], f32)
            nc.gpsimd.dma_start(out=t_sb, in_=t_dram.ap())
            tbm_sb = cpool.tile([H, 2 * W], f32)
            nc.gpsimd.dma_start(out=tbm_sb, in_=tbm_dram.ap())
            bias_sb = cpool.tile([H, 1], f32)
            nc.gpsimd.dma_start(out=bias_sb, in_=bias_dram.ap())

            for b in range(B_LOC):
                uv_psum = ppool.tile([H, 2 * W], f32)
                for s in range(2):
                    # channel-sum of this half: one strided DVE reduce over c
                    xs_s = xspool.tile([H, W], f32, tag=f"xs{s}")
                    nc.vector.reduce_sum(
                        xs_s,
                        halves[b][s].rearrange("h (c w) -> h w c", w=W),
                        axis=mybir.AxisListType.X,
                    )
                    # uv += xs_s.T @ [T | Bm.T]  (half-combine via PSUM accum)
                    nc.tensor.matmul(
                        uv_psum, xs_s, tbm_sb, start=(s == 0), stop=(s == 1)
                    )
                uv_sb = uvpool.tile([H, 2 * W], f32)
                nc.vector.tensor_copy(uv_sb, uv_psum)

                o_psum = ppool.tile([H, W], f32)
                nc.tensor.matmul(o_psum, uv_sb[:, 0:W], a_sb, start=True, stop=False)
                nc.tensor.matmul(
                    o_psum, uv_sb[:, W : 2 * W], t_sb, start=False, stop=True
                )

                o_sb = opool.tile([H, W], f32)
                nc.vector.tensor_scalar_add(o_sb, o_psum, bias_sb)
                nc.scalar.dma_start(out=out_ap[b], in_=o_sb)

    nc.compile()
    return nc


def _get_program():
    global _PROGRAM
    if _PROGRAM is None:
        _PROGRAM = _build_program()
    return _PROGRAM


def _build_consts(K, bias, lambda_c, lambda_a):
    K = np.asarray(K, np.float32)
    lc = float(np.asarray(lambda_c))
    la = float(np.asarray(lambda_a))
    a = (lc + la) * K.sum(axis=0)  # column sums -> horizontal taps
    b = (la - lc) * K.sum(axis=1)  # row sums -> vertical taps
    eye = np.eye(H, dtype=np.float32)
    up = np.eye(H, k=1, dtype=np.float32)
    dn = np.eye(H, k=-1, dtype=np.float32)
    T = eye + up + dn
    A = a[1] * eye + a[0] * up + a[2] * dn
    Bm = b[1] * eye + b[2] * up + b[0] * dn
    tbm = np.concatenate([T, Bm.T], axis=1)
    bias_col = np.full((H, 1), np.asarray(bias, np.float32).reshape(-1)[0], np.float32)
    return (
        np.ascontiguousarray(A, np.float32),
        np.ascontiguousarray(T, np.float32),
        np.ascontiguousarray(tbm, np.float32),
        bias_col,
    )


def kernel(x, K, bias, lambda_c, lambda_a, _trace=False):
    global LAST_RESULTS
    from concourse.bass_utils import run_bass_kernel_spmd

    x = np.asarray(x, np.float32)
    A, T, tbm, bias_col = _build_consts(K, bias, lambda_c, lambda_a)
    nc = _get_program()

    in_maps = []
    for core in range(N_CORES):
        shard = x[core * B_LOC : (core + 1) * B_LOC]  # [B_LOC, C, H, W]
        shard_t = np.ascontiguousarray(shard.transpose(2, 0, 1, 3))  # [H,B,C,W]
        in_maps.append(
            {"x": shard_t, "amat": A, "tmat": T, "tbm": tbm, "biascol": bias_col}
        )

    res = run_bass_kernel_spmd(
        nc, in_maps, core_ids=list(range(N_CORES)), trace=_trace
    )
    LAST_RESULTS = res
    out = np.concatenate([r["out"] for r in res.results], axis=0)
    return out.reshape(B, 1, H, W).astype(np.float32, copy=False)


# revision 9
# speedup vs baseline: 1.0287x; 1.0287x over previous
"""CommutatorConv2d kernel for Trainium2 (Bass/Tile), 8-core data-parallel.

Math: the reference's commutator/anticommutator conv reduces exactly to a
single-channel 3x3 conv on the channel-summed input:

    out[b] = T @ xs[b] @ A + Bm @ xs[b] @ T + bias,   xs = x.sum(axis=1)

where T is the 128x128 tridiagonal-ones matrix and A, Bm are tridiagonal
matrices built from K's column/row sums scaled by (lambda_c +/- lambda_a):
sum_{i,m} XK[...,i,m] = sum_{i,j} patch[i,j]*colsum(K)[j] and
sum_{j,i} KX[...,j,i] = sum_{m,i} patch[m,i]*rowsum(K)[m], so the effective
3x3 kernel is W[i,j] = a[j] + b[i], separable into a row-conv on the vertical
boxsum plus a col-conv on the horizontal boxsum = the two matrix sandwiches.

Layout: each core's batch shard is handed to the device as [H, B_loc, C, W]
(h-major) so every SBUF partition receives one long contiguous DRAM run per
DMA — 8KB descriptors instead of 512B ones, which is the difference between
~170 GB/s and ~358 GB/s on the HBM path. The device still streams the full
shard HBM->SBUF.

Per core (2 batches x 2 channel-halves): load half -> DVE strided reduce_sum
over c -> accumulate xs_half.T @ [T | Bm.T] into PSUM (the half-combine rides
the matmul accumulation) -> out = uv0.T @ A + uv1.T @ T in PSUM -> bias-add
fused into the PSUM->SBUF copy on DVE -> store. Loads ride the sync HWDGE
ring, stores the scalar ring, tiny constants the gpsimd SWDGE queue.
"""

import numpy as np

B, C, H, W = 16, 32, 128, 128
N_CORES = 8
B_LOC = B // N_CORES
HALF = C // 2

_PROGRAM = None
LAST_RESULTS = None


def _build_program():
    import concourse.mybir as mybir
    from concourse import bacc
    from concourse.bass import MemorySpace
    from concourse.tile import TileContext

    f32 = mybir.dt.float32
    nc = bacc.Bacc(
        "TRN2", target_bir_lowering=False, debug=False, num_devices=N_CORES
    )

    x_dram = nc.dram_tensor("x", (H, B_LOC, C, W), f32, kind="ExternalInput")
    # fused constants: [A | T | TBm | bias_col] as columns
    cm_dram = nc.dram_tensor("cmat", (H, 4 * W + 1), f32, kind="ExternalInput")
    out_dram = nc.dram_tensor("out", (B_LOC, H, W), f32, kind="ExternalOutput")

    x_ap = x_dram.ap()
    out_ap = out_dram.ap()

    with TileContext(nc) as tc:
        with (
            tc.tile_pool(name="consts", bufs=1) as cpool,
            tc.tile_pool(name="xpool", bufs=2) as xpool,
            tc.tile_pool(name="uvpool", bufs=2) as uvpool,
            tc.tile_pool(name="opool", bufs=2) as opool,
            tc.tile_pool(name="psum", bufs=2, space=MemorySpace.PSUM) as ppool,
        ):
            # Bulk x loads first (sync ring) so data streams immediately.
            halves = []  # [b][s] -> [H, HALF*W] tile
            for b in range(B_LOC):
                pair = []
                for s in range(2):
                    xh = xpool.tile([H, HALF * W], f32, tag=f"xh{s}")
                    nc.sync.dma_start(
                        out=xh.rearrange("h (c w) -> h c w", w=W),
                        in_=x_ap[:, b, s * HALF : (s + 1) * HALF, :],
                    )
                    pair.append(xh)
                halves.append(pair)

            # Fused constants via the gpsimd SWDGE queue (off the HWDGE ring).
            cm_sb = cpool.tile([H, 4 * W + 1], f32)
            nc.gpsimd.dma_start(out=cm_sb, in_=cm_dram.ap())
            a_sb = cm_sb[:, 0:W]
            t_sb = cm_sb[:, W : 2 * W]
            tbm_sb = cm_sb[:, 2 * W : 4 * W]
            bias_sb = cm_sb[:, 4 * W : 4 * W + 1]

            for b in range(B_LOC):
                uv_psum = ppool.tile([H, 2 * W], f32)
                for s in range(2):
                    # channel-sum of this half: in-place contiguous binary
                    # tree (peak-rate DVE). GpSimd takes the big first level
                    # of half 0 (lands early) to offload the vector engine.
                    xh = halves[b][s]
                    n = HALF * W
                    level = 0
                    while n > W:
                        n //= 2
                        eng = nc.gpsimd if (s == 0 and level == 0) else nc.vector
                        eng.tensor_add(xh[:, :n], xh[:, :n], xh[:, n : 2 * n])
                        level += 1
                    xs_s = xh[:, :W]
                    # uv += xs_s.T @ [T | Bm.T]  (half-combine via PSUM accum)
                    nc.tensor.matmul(
                        uv_psum, xs_s, tbm_sb, start=(s == 0), stop=(s == 1)
                    )
                uv_sb = uvpool.tile([H, 2 * W], f32)
                nc.vector.tensor_copy(uv_sb, uv_psum)

                o_psum = ppool.tile([H, W], f32)
                nc.tensor.matmul(o_psum, uv_sb[:, 0:W], a_sb, start=True, stop=False)
                nc.tensor.matmul(
                    o_psum, uv_sb[:, W : 2 * W], t_sb, start=False, stop=True
                )

                o_sb = opool.tile([H, W], f32)
                nc.vector.tensor_scalar_add(o_sb, o_psum, bias_sb)
                nc.scalar.dma_start(out=out_ap[b], in_=o_sb)

    nc.compile()
    return nc


def _get_program():
    global _PROGRAM
    if _PROGRAM is None:
        _PROGRAM = _build_program()
    return _PROGRAM


def _build_consts(K, bias, lambda_c, lambda_a):
    K = np.asarray(K, np.float32)
    lc = float(np.asarray(lambda_c))
    la = float(np.asarray(lambda_a))
    a = (lc + la) * K.sum(axis=0)  # column sums -> horizontal taps
    b = (la - lc) * K.sum(axis=1)  # row sums -> vertical taps
    eye = np.eye(H, dtype=np.float32)
    up = np.eye(H, k=1, dtype=np.float32)
    dn = np.eye(H, k=-1, dtype=np.float32)
    T = eye + up + dn
    A = a[1] * eye + a[0] * up + a[2] * dn
    Bm = b[1] * eye + b[2] * up + b[0] * dn
    bias_col = np.full((H, 1), np.asarray(bias, np.float32).reshape(-1)[0], np.float32)
    # fused [A | T | T | Bm.T | bias_col] -> [H, 4W+1]
    cm = np.concatenate([A, T, T, Bm.T, bias_col], axis=1)
    return np.ascontiguousarray(cm, np.float32)


def kernel(x, K, bias, lambda_c, lambda_a, _trace=False):
    global LAST_RESULTS
    from concourse.bass_utils import run_bass_kernel_spmd

    x = np.asarray(x, np.float32)
    cm = _build_consts(K, bias, lambda_c, lambda_a)
    nc = _get_program()

    in_maps = []
    for core in range(N_CORES):
        shard = x[core * B_LOC : (core + 1) * B_LOC]  # [B_LOC, C, H, W]
        shard_t = np.ascontiguousarray(shard.transpose(2, 0, 1, 3))  # [H,B,C,W]
        in_maps.append({"x": shard_t, "cmat": cm})

    res = run_bass_kernel_spmd(
        nc, in_maps, core_ids=list(range(N_CORES)), trace=_trace
    )
    LAST_RESULTS = res
    out = np.concatenate([r["out"] for r in res.results], axis=0)
    return out.reshape(B, 1, H, W).astype(np.float32, copy=False)
